# revision 35
# baseline (speedup 1.0000x reference)
"""Trainium2 Bass kernel v3 for nn_NbrAttn2 (neighbor cross-attention block).

Sharding: 8 cores = 4 batches x 2 kv-halves (unchanged from v2). Each core
computes attention for its batch over half the neighbors (KV = 8*512 = 4096)
for all 8 heads; softmax denominators z ([8,512] f32) are AllReduce-added
within each pair; each core normalizes its context partial, applies the
output projection, and writes a partial [T, D] output that the host pair-sums.

v3 changes (engine rebalance, attention is ACT-exp-bound):
- scores matmuls (K=DK=32) packed 2x via tile_position row-groups; ctx
  matmuls (M=HD=33) packed 2x via col-groups -> PE attention time ~4x down.
- attention pipelined per kv-chunk in 2-head groups: scores [128,1024] psum
  (2 banks, double-buffered) -> one exp per group -> [128,2048] masked mult
  on DVE (2x bf16 mode, mask broadcast over heads) -> col-packed ctx into 4
  persistent psum banks (head pair per bank, z row included at M=33).
- single ACT table set (natural_log_exp_and_others): LN rstd computed as
  exp(-0.5*ln(var+eps)) instead of Sqrt+reciprocal -> no table thrash.
- kT/qT stored as [128, KV]/[128, T] per 128-dim group (head h at partition
  32*(h%4)), making prep epilogues single big DVE ops and giving the packed
  scores their stationary layout for free.
- v += pe@Wv folded in 4-dim-AP DVE ops (4 instead of 16 per pair); mask
  DMA'd in the on-chip [128, KC*T] layout (128 contiguous 8KB descriptors)
  on the sync queue, freeing the gpsimd engine.
- z rows DMA'd straight from PSUM to the collective buffer; 1/z computed as
  [8,512] reciprocal (partition-parallel), broadcast per head with a tiny
  K=8 indicator matmul.
"""

import math

import numpy as np

B, T, N, D, H = 4, 512, 16, 256, 8
DK = D // H  # 32
HD = DK + 1  # ctx rows per head incl z
CTS, CN, CE = 6, 4, 3
TSE, AUXE = 192, 64
NCORES = 8
NBH = N // 2       # neighbors per core
KV = NBH * T       # 4096 kv positions per core
KC = KV // 128     # 32 kv chunks of 128

_CACHE = {}


def _pe_table() -> np.ndarray:
    # matches reference.pe_table numerics (fp32)
    pos = np.arange(T, dtype=np.float32)[:, None]
    div = np.exp(
        np.arange(0, D, 2, dtype=np.float32)
        * (np.float32(-np.log(np.float32(10000.0))) / np.float32(D))
    ).astype(np.float32)
    pe = np.zeros((T, D), dtype=np.float32)
    pe[:, 0::2] = np.sin(pos * div)
    pe[:, 1::2] = np.cos(pos * div)
    return pe


def build_nc(loop: int = 0, no_collective: bool = False, phases: str = "mlpa"):
    import concourse.bass as bass  # noqa: F401
    import concourse.mybir as mybir
    import concourse.tile as tile
    from concourse import bacc
    from concourse.masks import make_identity

    f32 = mybir.dt.float32
    bf16 = mybir.dt.bfloat16
    AF = mybir.ActivationFunctionType
    OP = mybir.AluOpType

    nc = bacc.Bacc()

    dp = nc.declare_dram_parameter
    maskt_h = dp("maskt", [128, KC * T], bf16, isOutput=False)  # on-chip layout
    xq_h = dp("xq", [128, 4 * D], f32, isOutput=False)    # 0.5*(x+pe), t-blocked
    # aux rows: 0-5 md, 32-35 na, 64-66 ea (32-aligned for matmul base rules)
    aux_h = dp("aux", [64 + CE, NBH * T], bf16, isOutput=False)
    # w1pack rows: 0-5 W_ts1, 32-35 W_a1, 64-66 W_e1
    w1pack_h = dp("w1pack", [64 + CE, D], bf16, isOutput=False)
    # wprep cols: wts2_0 0:192 | wts2_1 (rows 0:64) 192:384 | wa2 (rows 0:64)
    # 384:448 | we2_0 448:704 | we2_1 704:960
    wprep_h = dp("wprep", [128, 960], bf16, isOutput=False)
    # wattn cols: wq0 0:256 wq1 256:512 wk0 512:768 wk1 768:1024 wv0 1024:1280
    # wv1 1280:1536 wo0 1536:1792 wo1 1792:2048 wo_b(row0) 2048:2304
    # pewk0 2304:3328 pewk1 3328:4352 pewv4 4352:5376 ind8(rows0:8) 5376:5632
    wattn_h = dp("wattn", [128, 5632], bf16, isOutput=False)
    # fpack cols: 2i/2i+1 = bias_i lo/hi halves (i<6), 12/13 lng, 14/15 lnb
    fpack_h = dp("fpack", [128, 16], f32, isOutput=False)
    out_h = dp("out", [T, D], f32, isOutput=True)

    RG = [[0, 1], [2, 3], [4, 5], [6, 7]]
    NP = NBH // 2  # neighbor pairs

    do_m = "m" in phases
    do_l = "l" in phases
    do_p = "p" in phases
    do_a = "a" in phases
    do_epi = phases == "mlpa" or "z" in phases

    with tile.TileContext(nc, num_cores=NCORES) as tc:
        with (
            tc.tile_pool(name="const", bufs=1) as const,
            tc.tile_pool(name="big", bufs=1) as big,
            tc.tile_pool(name="prep", bufs=2) as prep,
            tc.tile_pool(name="ppool", bufs=3) as ppool,
            tc.tile_pool(name="dram", bufs=1, space="DRAM") as dram,
        ):
            # ---------------- constants ----------------
            ident = const.tile([128, 128], f32, name="ident")
            make_identity(nc, ident[:])
            ones_row = const.tile([1, T], bf16, name="ones_row")
            nc.vector.memset(ones_row[:], 1.0)
            actwarm = const.tile([1, T], bf16, name="actwarm")
            nc.scalar.activation(actwarm[:], ones_row[:], AF.Exp)
            # consolidated input DMAs, in consumption order, all on sync:
            # prep-critical first (fpack/w1pack/aux/wprep), then xq/wattn, mask
            fpack = const.tile([128, 16], f32, name="fpack")
            nc.sync.dma_start(out=fpack[:], in_=fpack_h[:])
            w1pack = const.tile([64 + CE, D], bf16, name="w1pack")
            nc.sync.dma_start(out=w1pack[:], in_=w1pack_h[:])
            w_ts1 = w1pack[0:CTS, 0:TSE]
            w_a1 = w1pack[32 : 32 + CN, 0:AUXE]
            w_e1 = w1pack[64 : 64 + CE, 0:D]
            aux_all = const.tile([64 + CE, NBH * T], bf16, name="aux_all")
            nc.sync.dma_start(out=aux_all[:], in_=aux_h[:])
            md_all = aux_all[0:CTS]
            na_all = aux_all[32 : 32 + CN]
            ea_all = aux_all[64 : 64 + CE]
            wprep = const.tile([128, 960], bf16, name="wprep")
            nc.sync.dma_start(out=wprep[:], in_=wprep_h[:])
            # double-buffered per-rep so the next body's xq DMA doesn't wait
            # on this body's residual reads
            xq_sbts = [const.tile([128, 4 * D], f32, name=f"xq_sbt{r}")
                       for r in range(2)]
            wattn = const.tile([128, 5632], bf16, name="wattn")
            nc.sync.dma_start(out=wattn[:], in_=wattn_h[:])

            w_ts2 = [wprep[:, 0:192], wprep[0:64, 192:384]]
            w_a2 = wprep[0:64, 384:448]
            w_e2 = [wprep[:, 448:704], wprep[:, 704:960]]
            w_q = [wattn[:, 0:256], wattn[:, 256:512]]
            w_k = [wattn[:, 512:768], wattn[:, 768:1024]]
            w_v = [wattn[:, 1024:1280], wattn[:, 1280:1536]]
            w_o = [wattn[:, 1536:1792], wattn[:, 1792:2048]]
            w_o_b = wattn[0:1, 2048:2304]
            pewk = [wattn[:, 2304 + c * 1024 : 2304 + (c + 1) * 1024] for c in range(2)]
            pewv4 = wattn[:, 4352:5376]
            ind8 = wattn[0:8, 5376:5632]

            class _BCol:
                """bias column i: fpack col 2i (dims 0:128) / 2i+1 (128:256)."""

                def __init__(self, i):
                    self.i = i

                def __getitem__(self, s):
                    lo, hi = s.start or 0, s.stop
                    c, r = divmod(lo, 128)
                    assert hi - lo <= 128 - r
                    cc = 2 * self.i + c
                    return fpack[r : r + (hi - lo), cc : cc + 1]

            # bias columns: 0=b_ts1, 1=b_ts2, 2=b_a (a1 0:64, a2 64:128),
            # 3=b_e1, 4=b_e2, 5=bq/sqrt(DK)
            bcol = [_BCol(i) for i in range(6)]
            lng = [fpack[:, 12 + c : 13 + c] for c in range(2)]
            lnb = [fpack[:, 14 + c : 15 + c] for c in range(2)]
            # ---------------- persistent big tensors ----------------
            mask_bf = big.tile([128, KC * T], bf16, name="mask_bf")
            # head h lives at partitions 32*(h%4) of group g=h//4
            kT_bf = [big.tile([128, KV], bf16, name=f"kT{g}") for g in range(2)]
            qT_bf = [big.tile([128, T], bf16, name=f"qT{g}") for g in range(2)]
            v_aug = big.tile([128, KC * H * HD], bf16, name="v_aug")
            xnT_bf = [big.tile([128, T], bf16, name=f"xnT{c}") for c in range(2)]

            va4 = v_aug.rearrange("p (c h e) -> p c h e", c=KC, h=H)
            nc.vector.memset(va4[:, :, :, DK : DK + 1], 1.0)
            va3 = v_aug.rearrange("p (c e) -> p c e", c=KC)  # [128, KC, 264]

            if do_a and not do_p:
                for t_ in kT_bf + qT_bf:
                    nc.vector.memset(t_[:], 0.5)
                nc.vector.memset(v_aug[:], 0.5)
                nc.vector.memset(va4[:, :, :, DK : DK + 1], 1.0)
            if do_a and not do_m:
                nc.vector.memset(mask_bf[:], 1.0)
            if do_epi and not do_l:
                for t_ in xq_sbts:
                    nc.vector.memset(t_[:], 0.0)

            for _rep in range(max(1, loop)):
                xq_sbt = xq_sbts[_rep % 2]
                xq_sb = [xq_sbt[:, t * D : (t + 1) * D] for t in range(4)]
                # ====== phase A+B: input DMAs, LN, q, per-pair prep ======
                with tc.tile_pool(name="pp", bufs=1, space="PSUM") as pp:
                    if do_l:
                        nc.sync.dma_start(out=xq_sbt[:], in_=xq_h[:])
                    if do_m:
                        for mc in range(4):
                            cs = slice(mc * 8 * T, (mc + 1) * 8 * T)
                            nc.sync.dma_start(out=mask_bf[:, cs], in_=maskt_h[:, cs])

                    def mm2(ps, terms, rows=slice(0, 128)):
                        """Accumulate matmul terms into both 512-halves of a
                        [128,1024] psum tile (one bank per matmul)."""
                        for j in range(2):
                            for idx, (lh, mv) in enumerate(terms):
                                nc.tensor.matmul(
                                    ps[rows, j * T : (j + 1) * T],
                                    lh, mv[:, j * T : (j + 1) * T],
                                    start=(idx == 0), stop=(idx == len(terms) - 1),
                                )

                    for np_ in range(NP if do_p else 0):
                        n0 = 2 * np_
                        nts2 = slice(n0 * T, (n0 + 2) * T)  # 2-neighbor slice
                        md2 = md_all[:, nts2]
                        na2 = na_all[:, nts2]
                        ea2 = ea_all[:, nts2]

                        # first layer: ts1/a1/e1 all independent (K<=32, and on
                        # distinct PE row-strips so they pack) -- emit all five
                        # matmul groups before any second-layer MM so the PE
                        # queue never head-of-line blocks on a relu drain.
                        ts1 = [prep.tile([128, 1024], bf16, name=f"ts1_{c}",
                                         tag=f"ts1{c}", bufs=2) for c in range(2)]
                        a1 = prep.tile([128, 1024], bf16, name="a1sb", tag="a1", bufs=2)
                        e1 = [prep.tile([128, 1024], bf16, name=f"e1_{c}",
                                        tag=f"e1{c}", bufs=2) for c in range(2)]
                        ps_ts1a = pp.tile([128, 1024], f32, name="ps_ts1a", tag="st", bufs=3)
                        mm2(ps_ts1a, [(w_ts1[:, 0:128], md2)])
                        ps_a1 = pp.tile([128, 1024], f32, name="ps_a1", tag="st", bufs=3)
                        mm2(ps_a1, [(w_a1[:], na2)], rows=slice(0, AUXE))
                        ps_e1a = pp.tile([128, 1024], f32, name="ps_e1a", tag="st", bufs=3)
                        mm2(ps_e1a, [(w_e1[:, 0:128], ea2)])
                        nc.scalar.activation(ts1[0][:], ps_ts1a[:], AF.Relu,
                                             bias=bcol[0][0:128])
                        ps_ts1b = pp.tile([128, 1024], f32, name="ps_ts1b", tag="st", bufs=3)
                        mm2(ps_ts1b, [(w_ts1[:, 128:TSE], md2)], rows=slice(0, TSE - 128))
                        nc.scalar.activation(
                            a1[0:AUXE, :], ps_a1[0:AUXE, :], AF.Relu, bias=bcol[2][0:AUXE])
                        ps_e1b = pp.tile([128, 1024], f32, name="ps_e1b", tag="st", bufs=3)
                        mm2(ps_e1b, [(w_e1[:, 128:256], ea2)])
                        nc.scalar.activation(e1[0][:], ps_e1a[:], AF.Relu,
                                             bias=bcol[3][0:128])
                        nc.scalar.activation(
                            ts1[1][0 : TSE - 128, :], ps_ts1b[0 : TSE - 128, :],
                            AF.Relu, bias=bcol[0][128:TSE])
                        nc.scalar.activation(e1[1][:], ps_e1b[:], AF.Relu,
                                             bias=bcol[3][128:256])

                        # second layer: nbr = [ts2 (192) ; a2 (64)], e2
                        nbr = [prep.tile([128, 1024], bf16, name=f"nbr_{c}",
                                         tag=f"nbr{c}", bufs=2) for c in range(2)]
                        ps = pp.tile([128, 1024], f32, name="ps_nbr0", tag="st", bufs=3)
                        mm2(ps, [(w_ts2[0][:, 0:128], ts1[0]),
                                 (w_ts2[1][0:64, 0:128], ts1[1][0:64, :])])
                        nc.vector.tensor_scalar_add(nbr[0][:], ps[:], bcol[1][0:128])
                        ps = pp.tile([128, 1024], f32, name="ps_nbr1", tag="st", bufs=3)
                        mm2(ps, [(w_ts2[0][:, 128:TSE], ts1[0]),
                                 (w_ts2[1][0:64, 128:TSE], ts1[1][0:64, :])],
                            rows=slice(0, 64))
                        nc.vector.tensor_scalar_add(
                            nbr[1][0:64, :], ps[0:64, :], bcol[1][128:TSE])
                        ps = pp.tile([128, 1024], f32, name="ps_a2", tag="st", bufs=3)
                        mm2(ps, [(w_a2, a1[0:AUXE, :])], rows=slice(0, AUXE))
                        nc.scalar.activation(
                            nbr[1][64:128, :], ps[0:AUXE, :], AF.Identity,
                            bias=bcol[2][64:128])
                        e2 = [prep.tile([128, 1024], bf16, name=f"e2_{c}",
                                        tag=f"e2{c}", bufs=2) for c in range(2)]
                        for c in range(2):
                            cs = slice(c * 128, (c + 1) * 128)
                            ps = pp.tile([128, 1024], f32, name="ps_e2", tag="st", bufs=3)
                            mm2(ps, [(w_e2[0][:, cs], e1[0]), (w_e2[1][:, cs], e1[1])])
                            nc.scalar.activation(e2[c][:], ps[:], AF.Identity,
                                                 bias=bcol[4][cs])

                        # keysT = nbr * e2 (pe+bias via pewk const)
                        keys = [prep.tile([128, 1024], bf16, name=f"keys_{c}",
                                          tag=f"keys{c}", bufs=2) for c in range(2)]
                        nc.gpsimd.tensor_tensor(keys[0][:], nbr[0][:], e2[0][:], OP.mult)
                        nc.gpsimd.tensor_tensor(keys[1][:], nbr[1][:], e2[1][:], OP.mult)

                        # v rows (before kT: only needs nbr, overlaps the Pool
                        # keys-mult latency): v = nbr^T Wv (+ pewv 4-dim AP add)
                        for i in range(2):
                            for j2 in range(2):
                                ps = pp.tile([128, 512], f32, name="ps_v", tag="v", bufs=2)
                                for jj in range(2):
                                    j = 2 * j2 + jj
                                    ts_ = slice(i * 512 + j * 128, i * 512 + (j + 1) * 128)
                                    nc.tensor.matmul(
                                        ps[:, jj * D : (jj + 1) * D], nbr[0][:, ts_],
                                        w_v[0], start=True, stop=False)
                                    nc.tensor.matmul(
                                        ps[:, jj * D : (jj + 1) * D], nbr[1][:, ts_],
                                        w_v[1], start=False, stop=True)
                                kc0 = (n0 + i) * 4 + 2 * j2
                                nc.vector.tensor_tensor(
                                    va4[:, kc0 : kc0 + 2, :, 0:DK],
                                    ps[:].rearrange("p (j h e) -> p j h e", j=2, h=H),
                                    pewv4[:, 2 * j2 * D : (2 * j2 + 2) * D].rearrange(
                                        "p (j h e) -> p j h e", j=2, h=H),
                                    OP.add,
                                )

                        # kT = Wk^T keys + pewk^T -> [128, KV] per group
                        for g in range(2):
                            gs = slice(g * 128, (g + 1) * 128)
                            ps = pp.tile([128, 1024], f32, name="ps_kt", tag="st", bufs=3)
                            mm2(ps, [(w_k[0][:, gs], keys[0]), (w_k[1][:, gs], keys[1])])
                            nc.vector.tensor_tensor(
                                kT_bf[g][:, nts2], ps[:], pewk[g], OP.add)

                    # LN + q after prep in program order: prep matmuls head the
                    # PE queue (no LN-chain head-of-line stall); LN runs on
                    # DVE/ACT slack during prep. var via ACT Square+accum,
                    # 1/sqrt on DVE (magic-seed + 2 Newton) so ACT stays on the
                    # single exp_and_others table set.
                    xcs = []
                    var4 = prep.tile([128, 4], f32, name="ln_var4", tag="lnvar", bufs=1)
                    for t in range(4 if do_l else 0):
                        xt = xq_sb[t]
                        mu = prep.tile([128, 1], f32, name="ln_mu", tag="lnmu", bufs=4)
                        nc.vector.tensor_reduce(
                            mu[:], xt, mybir.AxisListType.X, OP.add
                        )
                        nc.vector.tensor_scalar_mul(mu[:], mu[:], 1.0 / D)
                        xc = prep.tile([128, D], f32, name="ln_xc", tag=f"lnxc{t}",
                                       bufs=1)
                        nc.vector.tensor_scalar(xc[:], xt, mu[:], None, OP.subtract)
                        sq = prep.tile([128, D], f32, name="ln_sq", tag="lnsq", bufs=2)
                        nc.scalar.activation(sq[:], xc[:], AF.Square,
                                             accum_out=var4[:, t : t + 1])
                        xcs.append(xc)
                    if do_l:
                        # v = var/D + eps; rstd = rsqrt(v) via bit trick + Newton
                        v4 = prep.tile([128, 4], f32, name="ln_v4", tag="lnv4", bufs=1)
                        nc.vector.tensor_scalar(v4[:], var4[:], 1.0 / D, 0.25e-6,
                                                OP.mult, OP.add)
                        y0i = prep.tile([128, 4], mybir.dt.int32, name="ln_y0i",
                                        tag="lnyi", bufs=1)
                        nc.vector.tensor_scalar(
                            y0i[:], v4[:].bitcast(mybir.dt.int32), 1, None,
                            OP.logical_shift_right)
                        # 0x5f3759df - (i >> 1): subtract via reverse operand
                        y0 = prep.tile([128, 4], f32, name="ln_y0", tag="lny", bufs=3)
                        nc.vector.tensor_scalar(
                            y0[:].bitcast(mybir.dt.int32), y0i[:], -1, 0x5F3759DF,
                            OP.mult, OP.add)
                        yy = y0
                        for _it in range(2):
                            t1 = prep.tile([128, 4], f32, name="ln_t1", tag="lnt",
                                           bufs=2)
                            nc.vector.tensor_tensor(t1[:], yy[:], yy[:], OP.mult)
                            nc.vector.tensor_tensor(t1[:], t1[:], v4[:], OP.mult)
                            nc.vector.tensor_scalar(t1[:], t1[:], -0.5, 1.5,
                                                    OP.mult, OP.add)
                            y1 = prep.tile([128, 4], f32, name="ln_y1", tag="lny",
                                           bufs=3)
                            nc.vector.tensor_tensor(y1[:], yy[:], t1[:], OP.mult)
                            yy = y1
                    for t in range(4 if do_l else 0):
                        xn0 = prep.tile([128, D], f32, name="ln_xn0", tag="lnw", bufs=2)
                        nc.vector.tensor_scalar_mul(xn0[:], xcs[t][:], yy[:, t : t + 1])
                        for c in range(2):
                            tp = pp.tile([128, 512], f32, name="tp", tag="v", bufs=2)
                            nc.tensor.transpose(
                                tp[:, 0:128], xn0[:, c * 128 : (c + 1) * 128], ident[:]
                            )
                            nc.vector.tensor_scalar(
                                xnT_bf[c][:, t * 128 : (t + 1) * 128],
                                tp[:, 0:128], lng[c], lnb[c], OP.mult, OP.add,
                            )
                    for g in range(2 if do_l else 0):
                        qp = pp.tile([128, 1024], f32, name="qp", tag="st", bufs=3)
                        gs = slice(g * 128, (g + 1) * 128)
                        nc.tensor.matmul(qp[:, 0:T], w_q[0][:, gs], xnT_bf[0][:],
                                         start=True, stop=False)
                        nc.tensor.matmul(qp[:, 0:T], w_q[1][:, gs], xnT_bf[1][:],
                                         start=False, stop=True)
                        nc.vector.tensor_scalar_add(
                            qT_bf[g][:], qp[:, 0:T], bcol[5][g * 128 : (g + 1) * 128]
                        )

                # ====== phase C: pipelined packed attention ======
                if do_a:
                    with tc.tile_pool(name="pcx", bufs=1, space="PSUM") as pcx:
                        # bank j: head 2j at rows 0:33, head 2j+1 at rows 64:97
                        cxb = [pcx.tile([128, T], f32, name=f"cxb{j}")
                               for j in range(4)]
                        def ctx_mms(kc, pm):
                            for j in range(4):
                                for i in range(2):
                                    h = 2 * j + i
                                    nc.tensor.matmul(
                                        cxb[j][64 * i : 64 * i + HD, :],
                                        va3[:, kc, h * HD : (h + 1) * HD],
                                        pm[:, h * T : (h + 1) * T],
                                        start=(kc == 0), stop=(kc == KC - 1),
                                    )

                        with tc.tile_pool(name="psc", bufs=1, space="PSUM") as psc:
                            pm_prev = None
                            for kc in range(KC):
                                p0 = ppool.tile([128, H * T], bf16, name="p0",
                                                tag="p0", bufs=3)
                                pm = ppool.tile([128, H * T], bf16, name="pm",
                                                tag="pm", bufs=2)
                                for G in range(4):
                                    g, pr = divmod(G, 2)
                                    sp = psc.tile([128, 2 * T], f32, name="sp",
                                                  tag="sp", bufs=2)
                                    for i in range(2):
                                        po = 64 * pr + 32 * i
                                        nc.tensor.matmul(
                                            sp[:, i * T : (i + 1) * T],
                                            kT_bf[g][po : po + 32,
                                                     kc * 128 : (kc + 1) * 128],
                                            qT_bf[g][po : po + 32, :],
                                            start=True, stop=True,
                                            tile_position=(po, 0),
                                        )
                                    nc.scalar.activation(
                                        p0[:, G * 2 * T : (G + 1) * 2 * T], sp[:], AF.Exp
                                    )
                                mbc = mask_bf[:, kc * T : (kc + 1) * T].rearrange(
                                    "p (o t) -> p o t", o=1
                                ).to_broadcast((128, 4, T))
                                for half in range(2):
                                    hs = slice(half * 4 * T, (half + 1) * 4 * T)
                                    nc.vector.tensor_tensor(
                                        pm[:, hs].rearrange("p (o t) -> p o t", o=4),
                                        p0[:, hs].rearrange("p (o t) -> p o t", o=4),
                                        mbc, OP.mult,
                                    )
                                # ctx lags one chunk so next chunk's scores sit
                                # ahead of it in the PE queue (keeps ACT fed)
                                if pm_prev is not None:
                                    ctx_mms(kc - 1, pm_prev)
                                pm_prev = pm
                            ctx_mms(KC - 1, pm_prev)

                        if do_epi:
                            # drain ctx banks to SBUF immediately (split over
                            # DVE/ACT) so all PSUM frees before the collective;
                            # next body's prep can overlap the epilogue chain.
                            cxs = [ppool.tile([128, T], bf16, name=f"cxs{j}",
                                              tag="cxs", bufs=4) for j in range(4)]
                            for j in range(4):
                                if j % 2 == 0:
                                    nc.vector.tensor_copy(cxs[j][:], cxb[j][:])
                                else:
                                    nc.scalar.activation(cxs[j][:], cxb[j][:],
                                                         AF.Identity)

                    # ====== phase D: z AllReduce + epilogue (PSUM-light) ======
                    if do_epi:
                        with tc.tile_pool(name="px", bufs=2, space="PSUM") as px:
                            cc_in = dram.tile([H * T], f32, name="cc_in")
                            cc_out = dram.tile([H * T], f32, name="cc_out")
                            # z row for head h at partition 32*(h%4), col-half h//4
                            z_tile = big.tile([128, 2 * T], f32, name="z_tile")
                            for j in range(4):
                                for i in range(2):
                                    h = 2 * j + i
                                    zdst = z_tile[32 * (h % 4) : 32 * (h % 4) + 1,
                                                  (h // 4) * T : (h // 4 + 1) * T]
                                    zsrc = cxs[j][64 * i + DK : 64 * i + DK + 1, :]
                                    if h % 2 == 0:
                                        nc.vector.tensor_copy(zdst, zsrc)
                                    else:
                                        nc.scalar.activation(zdst, zsrc, AF.Identity)
                            zv = z_tile.rearrange("(a b) (c t) -> a b c t", b=32, c=2)
                            nc.sync.dma_start(
                                out=cc_in.rearrange("(c a t) -> a c t", a=4, c=2),
                                in_=zv[:, 0, :, :],
                            )
                            if no_collective:
                                nc.sync.dma_start(out=cc_out[:], in_=cc_in[:])
                            else:
                                nc.gpsimd.collective_compute(
                                    "AllReduce", OP.add, replica_groups=RG,
                                    ins=[cc_in[:]], outs=[cc_out[:]],
                                )
                            zg8 = big.tile([8, T], f32, name="zg8")
                            nc.sync.dma_start(
                                out=zg8[:],
                                in_=cc_out.rearrange("(h t) -> h t", h=H),
                            )
                            rzf8 = big.tile([8, T], bf16, name="rzf8")
                            ctxn = [big.tile([128, T], bf16, name=f"ctxn{c}")
                                    for c in range(2)]
                            with nc.allow_low_precision(reason="1/z bf16; tol 2e-2"):
                                nc.vector.reciprocal(rzf8[:], zg8[:])
                                for j in range(4):
                                    # 1/z broadcast rows in ctx-bank layout
                                    # (head 2j at rows 0:32, head 2j+1 at 64:96)
                                    # so the SBUF mult bases line up with cxs
                                    bc = px.tile([128, T], f32, name="bc", tag="px")
                                    for i in range(2):
                                        h = 2 * j + i
                                        nc.tensor.matmul(
                                            bc[64 * i : 64 * i + DK, :],
                                            ind8[:, h * DK : (h + 1) * DK],
                                            rzf8[:], start=True, stop=True,
                                            tile_position=(0, 64 * i),
                                        )
                                    bcs = prep.tile([128, T], bf16, name="bcs",
                                                    tag="bcs", bufs=2)
                                    nc.vector.tensor_copy(bcs[:], bc[:])
                                    for i in range(2):
                                        h = 2 * j + i
                                        c4, r4 = divmod(h, 4)
                                        nc.vector.tensor_tensor(
                                            ctxn[c4][r4 * DK : (r4 + 1) * DK, :],
                                            cxs[j][64 * i : 64 * i + DK, :],
                                            bcs[64 * i : 64 * i + DK, :], OP.mult)
                            for t in range(4):
                                ts_ = slice(t * 128, (t + 1) * 128)
                                op_ = px.tile([128, D], f32, name="op", tag="po")
                                nc.tensor.matmul(op_[:], ctxn[0][:, ts_], w_o[0],
                                                 start=True, stop=False)
                                nc.tensor.matmul(op_[:], ctxn[1][:, ts_], w_o[1],
                                                 start=False, stop=False)
                                nc.tensor.matmul(op_[:], ones_row[0:1, ts_], w_o_b,
                                                 start=False, stop=True)
                                ot = prep.tile([128, D], f32, name="out_sb",
                                               tag="lnw", bufs=2)
                                nc.vector.tensor_add(ot[:], op_[:], xq_sb[t])
                                nc.sync.dma_start(out=out_h[ts_, :], in_=ot[:])

    nc.finalize()
    return nc


def _host_inputs(inputs):
    """Build the 8 per-core input maps from full inputs."""
    import ml_dtypes

    bf16 = ml_dtypes.bfloat16
    pe = _pe_table()
    sc = np.float32(1.0 / math.sqrt(DK))

    w = {k: np.asarray(v, dtype=np.float32) if np.asarray(v).dtype != np.int32
         else np.asarray(v) for k, v in inputs.items()}

    def pad_col(v):
        out = np.zeros((D, 1), np.float32)
        out[: v.shape[0], 0] = v
        return out

    biases = np.stack([
        pad_col(w["b_ts1"]),
        pad_col(w["b_ts2"]),
        pad_col(np.concatenate([w["b_a1"], w["b_a2"]])),
        pad_col(w["b_e1"]),
        pad_col(w["b_e2"]),
        pad_col(w["bq"] * sc),
    ])

    w1pack = np.zeros((64 + CE, D), np.float32)
    w1pack[0:CTS, 0:TSE] = w["W_ts1"]
    w1pack[32 : 32 + CN, 0:AUXE] = w["W_a1"]
    w1pack[64 : 64 + CE, 0:D] = w["W_e1"]

    pewv = (pe @ w["Wv"] + w["bv"]).astype(np.float32)  # [T, D]
    pewkT = np.tile((pe @ w["Wk"] + w["bk"]).T, (1, 2)).astype(np.float32)  # [D, 2T]

    wprep = np.zeros((128, 960), np.float32)
    wprep[:, 0:192] = w["W_ts2"][0:128]
    wprep[0:64, 192:384] = w["W_ts2"][128:192]
    wprep[0:64, 384:448] = w["W_a2"]
    wprep[:, 448:704] = w["W_e2"][0:128]
    wprep[:, 704:960] = w["W_e2"][128:256]

    def build_wattn(bias_on):
        a = np.zeros((128, 5632), np.float32)
        wq = w["Wq"] * sc
        a[:, 0:256] = wq[0:128]
        a[:, 256:512] = wq[128:256]
        a[:, 512:768] = w["Wk"][0:128]
        a[:, 768:1024] = w["Wk"][128:256]
        a[:, 1024:1280] = w["Wv"][0:128]
        a[:, 1280:1536] = w["Wv"][128:256]
        a[:, 1536:1792] = w["Wo"][0:128]
        a[:, 1792:2048] = w["Wo"][128:256]
        if bias_on:
            a[0, 2048:2304] = w["bo"]
        a[:, 2304:3328] = pewkT[0:128]
        a[:, 3328:4352] = pewkT[128:256]
        a[:, 4352:5376] = pewv.reshape(4, 128, D).transpose(1, 0, 2).reshape(128, 4 * D)
        a[0:8, 5376:5632] = np.kron(np.eye(H, dtype=np.float32),
                                    np.ones((1, DK), np.float32))
        return a.astype(bf16)

    fpack = np.zeros((128, 16), np.float32)
    for i in range(6):
        fpack[:, 2 * i] = biases[i][0:128, 0]
        fpack[:, 2 * i + 1] = biases[i][128:256, 0]
    fpack[:, 12] = w["ln_g"][0:128]
    fpack[:, 13] = w["ln_g"][128:256]
    fpack[:, 14] = w["ln_b"][0:128]
    fpack[:, 15] = w["ln_b"][128:256]

    shared = {
        "w1pack": w1pack.astype(bf16),
        "wprep": wprep.astype(bf16),
        "fpack": fpack,
    }
    wattn_even = build_wattn(True)
    wattn_odd = build_wattn(False)

    in_maps = []
    for c in range(NCORES):
        b, half = divmod(c, 2)
        n0 = half * NBH
        m = dict(shared)
        # half-scaled: LN is scale-invariant (with eps/4 on device) and both
        # pair cores add it as residual, so the host pair-sum restores 1.0x.
        xqf = (0.5 * (w["x"][b] + pe)).astype(np.float32)
        m["xq"] = np.ascontiguousarray(
            xqf.reshape(4, 128, D).transpose(1, 0, 2).reshape(128, 4 * D)
        )
        aux = np.zeros((64 + CE, NBH * T), np.float32)
        aux[0:CTS] = w["masked_data"][b, n0 : n0 + NBH].transpose(1, 0, 2).reshape(CTS, -1)
        aux[32 : 32 + CN] = w["node_aux"][b, n0 : n0 + NBH].transpose(1, 0, 2).reshape(CN, -1)
        aux[64 : 64 + CE] = w["edge_aux"][b, n0 : n0 + NBH].transpose(1, 0, 2).reshape(CE, -1)
        m["aux"] = aux.astype(bf16)
        # mask in on-chip layout: maskt[p, c*T + t] = mask[c*128+p, t]
        msk = w["attention_mask"][b, :, half * KV : (half + 1) * KV].T  # [KV, T]
        m["maskt"] = np.ascontiguousarray(
            msk.reshape(KC, 128, T).transpose(1, 0, 2).reshape(128, KC * T)
        ).astype(bf16)
        m["wattn"] = wattn_even if half == 0 else wattn_odd
        in_maps.append(m)
    return in_maps


def _get_nc():
    if "nc" not in _CACHE:
        _CACHE["nc"] = build_nc()
    return _CACHE["nc"]


def kernel(**inputs) -> np.ndarray:
    from concourse.bass_utils import run_bass_kernel_spmd

    nc = _get_nc()
    in_maps = _host_inputs(inputs)
    res = run_bass_kernel_spmd(nc, in_maps, list(range(NCORES)))
    out = np.stack(
        [res.results[2 * b]["out"] + res.results[2 * b + 1]["out"] for b in range(B)],
        axis=0,
    )
    return out.astype(np.float32)
